# revision 13
# baseline (speedup 1.0000x reference)
"""Trainium2 Bass kernel for GQA multi-head attention (RoPE + padding|causal mask).

Sharding: 8 cores = 2 (batch) x 4 (KV-head pairs). Each core computes
attention for its 2 KV heads / 4 query heads of one batch element, then an
AllGather within each batch group reshards the per-head attention outputs so
every core computes a disjoint 256-column slice of the output projection.
The host only slices/permutes inputs and concatenates output slices.

Per-core device algorithm (all in transposed layout, no on-device transposes):
  qT/kT = W^T x^T with host-permuted weight columns so RoPE pairs are
  deinterleaved into 32-row blocks (RoPE = 6 elementwise ops per m-tile).
  scoresT[sk, sq] = kT.T @ qT (K=64 matmuls packed 2-at-a-time into PE
  row-groups 0-63/64-127). exp on ScalarE reads PSUM directly with the
  1/sqrt(dk) scale fused. No max subtraction (scores are provably small).
  Mask: V carries a ones column (softmax denominator rides along row 64 of
  the AV matmul), the strictly-future region uses a pad-masked V copy as
  stationary operand, diagonal 128x128 blocks get an elementwise 0/1 mask.
"""

import sys

if "/opt/trn_rl_repo" not in sys.path:
    sys.path.insert(0, "/opt/trn_rl_repo")

import numpy as np

import concourse.mybir as mybir
import concourse.tile as tile
from concourse import bacc
from concourse.bass_utils import run_bass_kernel_spmd

B, S, D = 2, 2048, 1024
H_Q, H_KV, DK, DV = 16, 8, 64, 64
N_CORES = 8
P = 128
FP = mybir.dt.float32
SCALE = 1.0 / 8.0  # 1/sqrt(DK)
NSLICE = D // 4  # output-projection column slice per core (256)
NK = D // P  # 8 k-tiles
NSK = S // P  # 16 sk tiles


def build_nc():
    nc = bacc.Bacc("TRN2", target_bir_lowering=False, debug=False,
                   num_devices=N_CORES)

    xT = nc.dram_tensor("xT", [D, S], FP, kind="ExternalInput")
    wq = nc.dram_tensor("wq", [D, 256], FP, kind="ExternalInput")
    wk = nc.dram_tensor("wk", [D, 128], FP, kind="ExternalInput")
    wv = nc.dram_tensor("wv", [D, 130], FP, kind="ExternalInput")
    ev = nc.dram_tensor("ev", [1, 130], FP, kind="ExternalInput")
    wo = nc.dram_tensor("wo", [H_Q * DV, NSLICE], FP, kind="ExternalInput")
    bo = nc.dram_tensor("bo", [1, NSLICE], FP, kind="ExternalInput")
    cosA = nc.dram_tensor("cosA", [P, S], FP, kind="ExternalInput")
    sinA = nc.dram_tensor("sinA", [P, S], FP, kind="ExternalInput")
    tri = nc.dram_tensor("tri", [P, P], FP, kind="ExternalInput")
    pad2 = nc.dram_tensor("pad2", [P, 16], FP, kind="ExternalInput")
    ones = nc.dram_tensor("ones", [1, 512], FP, kind="ExternalInput")
    bq2 = nc.dram_tensor("bq2", [1, 2 * P], FP, kind="ExternalInput")
    bk1 = nc.dram_tensor("bk1", [1, P], FP, kind="ExternalInput")
    outp = nc.dram_tensor("outp", [S, NSLICE], FP, kind="ExternalOutput")

    Exp = mybir.ActivationFunctionType.Exp

    with tile.TileContext(nc) as tc:
        with (
            tc.tile_pool(name="persist", bufs=1) as pp,
            tc.tile_pool(name="dram", bufs=1, space="DRAM") as dp,
        ):
            xT_sb = pp.tile([P, NK * S], FP, tag="xT")
            wo_sb = pp.tile([P, NK * NSLICE], FP, tag="wo")
            tri_sb = pp.tile([P, P], FP, tag="tri")
            pad_sb = pp.tile([P, 16], FP, tag="pad")
            ones_sb = pp.tile([1, 512], FP, tag="ones")
            bo_sb = pp.tile([1, NSLICE], FP, tag="bo")
            qTA = pp.tile([P, S], FP, tag="qTA")
            qTB = pp.tile([P, S], FP, tag="qTB")
            kT0 = pp.tile([P, S], FP, tag="kT0")
            kT1 = pp.tile([P, S], FP, tag="kT1")
            vv = pp.tile([P, NSK * 130], FP, tag="vv")
            vp = pp.tile([P, NSK * 130], FP, tag="vp")

            for kk in range(NK):
                nc.sync.dma_start(xT_sb[:, kk * S:(kk + 1) * S],
                                  xT[kk * P:(kk + 1) * P, :])
                nc.sync.dma_start(wo_sb[:, kk * NSLICE:(kk + 1) * NSLICE],
                                  wo[kk * P:(kk + 1) * P, :])
            nc.sync.dma_start(tri_sb[:], tri[:])
            nc.sync.dma_start(pad_sb[:], pad2[:])
            nc.sync.dma_start(ones_sb[:], ones[:])
            nc.sync.dma_start(bo_sb[:], bo[:])

            # ---------- phase 1: QKV projections + RoPE ----------
            with (
                tc.tile_pool(name="ps_qkv", bufs=1, space="PSUM") as pq,
                tc.tile_pool(name="ps_v", bufs=2, space="PSUM") as pv,
                tc.tile_pool(name="rope_tmp", bufs=1) as rt,
                tc.tile_pool(name="p1", bufs=1) as p1,
            ):
                wq_sb = p1.tile([P, NK * 256], FP, tag="wq")
                wk_sb = p1.tile([P, NK * 128], FP, tag="wk")
                wv_sb = p1.tile([P, NK * 130], FP, tag="wv")
                cos_sb = p1.tile([P, S], FP, tag="cos")
                sin_sb = p1.tile([P, S], FP, tag="sin")
                ev_sb = p1.tile([1, 130], FP, tag="ev")
                bq_sb = p1.tile([1, 2 * P], FP, tag="bq")
                bk_sb = p1.tile([1, P], FP, tag="bk")
                for kk in range(NK):
                    nc.sync.dma_start(wq_sb[:, kk * 256:(kk + 1) * 256],
                                      wq[kk * P:(kk + 1) * P, :])
                    nc.sync.dma_start(wk_sb[:, kk * 128:(kk + 1) * 128],
                                      wk[kk * P:(kk + 1) * P, :])
                    nc.sync.dma_start(wv_sb[:, kk * 130:(kk + 1) * 130],
                                      wv[kk * P:(kk + 1) * P, :])
                nc.sync.dma_start(cos_sb[:], cosA[:])
                nc.sync.dma_start(sin_sb[:], sinA[:])
                nc.sync.dma_start(ev_sb[:], ev[:])
                nc.sync.dma_start(bq_sb[:], bq2[:])
                nc.sync.dma_start(bk_sb[:], bk1[:])
                def qkv_mtile(w_sb, wcols, mt, bias_ap, dest_ops):
                    ps = pq.tile([P, S], FP, tag="qkv")
                    for n in range(S // 512):
                        nc.tensor.matmul(ps[:, n * 512:(n + 1) * 512],
                                         bias_ap, ones_sb[:, 0:512],
                                         start=True, stop=False)
                    for kk in range(NK):
                        lhsT = w_sb[:, kk * wcols + mt * P:
                                    kk * wcols + (mt + 1) * P]
                        for n in range(S // 512):
                            nc.tensor.matmul(
                                ps[:, n * 512:(n + 1) * 512], lhsT,
                                xT_sb[:, kk * S + n * 512:kk * S + (n + 1) * 512],
                                start=False, stop=(kk == NK - 1))
                    t_sb = rt.tile([P, S], FP, tag="ropeT")
                    s_sb = rt.tile([P, S], FP, tag="ropeS")
                    ss = rt.tile([P, S], FP, tag="ropeSS")
                    nc.vector.tensor_mul(t_sb[:], ps[:], cos_sb[:])
                    nc.vector.tensor_mul(s_sb[:], ps[:], sin_sb[:])
                    # swap 32-row blocks pairwise so every op is lane-aligned
                    for blk in range(4):
                        r0, rs = blk * 32, (blk ^ 1) * 32
                        nc.sync.dma_start(ss[r0:r0 + 32, :],
                                          s_sb[rs:rs + 32, :])
                    for blk, dst in dest_ops:
                        r0 = blk * 32
                        if blk % 2 == 0:  # x1 block: o1 = x1*c - x2*s
                            nc.vector.tensor_sub(dst, t_sb[r0:r0 + 32, :],
                                                 ss[r0:r0 + 32, :])
                        else:  # x2 block: o2 = x1*s + x2*c
                            nc.vector.tensor_add(dst, ss[r0:r0 + 32, :],
                                                 t_sb[r0:r0 + 32, :])

                qkv_mtile(wq_sb, 256, 0, bq_sb[:, 0:P],
                          [(i, qTA[i * 32:(i + 1) * 32, :]) for i in range(4)])
                qkv_mtile(wq_sb, 256, 1, bq_sb[:, P:2 * P],
                          [(i, qTB[i * 32:(i + 1) * 32, :]) for i in range(4)])
                ktmp = rt.tile([P, S], FP, tag="ktmp")
                qkv_mtile(wk_sb, 128, 0, bk_sb[0:1, :],
                          [(i, ktmp[i * 32:(i + 1) * 32, :]) for i in range(4)])
                nc.sync.dma_start(kT0[0:64, :], ktmp[0:64, :])
                nc.sync.dma_start(kT0[64:128, :], ktmp[0:64, :])
                nc.sync.dma_start(kT1[0:64, :], ktmp[64:128, :])
                nc.sync.dma_start(kT1[64:128, :], ktmp[64:128, :])

                for i in range(NSK):
                    ps_v = pv.tile([P, 130], FP, tag="v")
                    nc.tensor.matmul(ps_v[:], ones_sb[:, 0:P], ev_sb[:],
                                     start=True, stop=False)
                    for kk in range(NK):
                        nc.tensor.matmul(
                            ps_v[:],
                            xT_sb[:, kk * S + i * P:kk * S + (i + 1) * P],
                            wv_sb[:, kk * 130:(kk + 1) * 130],
                            start=False, stop=(kk == NK - 1))
                    nc.vector.tensor_copy(vv[:, i * 130:(i + 1) * 130], ps_v[:])
                    nc.vector.tensor_scalar_mul(vp[:, i * 130:(i + 1) * 130],
                                                ps_v[:], pad_sb[:, i:i + 1])

            # ---------- phase 2: attention ----------
            ag_in = dp.tile([2 * P, S], FP, tag="agin")
            ag_out = dp.tile([8 * P, S], FP, tag="agout")
            with (
                tc.tile_pool(name="ps_attn", bufs=1, space="PSUM") as pa,
                tc.tile_pool(name="exp_pool", bufs=2) as epool,
                tc.tile_pool(name="norm_pool", bufs=1) as npo,
                tc.tile_pool(name="st_pool", bufs=2) as stp,
                tc.tile_pool(name="keep_pool", bufs=2) as kpo,
            ):
                for g, (qT, kT) in enumerate(((qTA, kT0), (qTB, kT1))):
                    for sqh in range(2):
                        o = sqh * 1024
                        av = [pa.tile([65, 1024], FP, tag=f"av{h}", name=f"av{h}")
                              for h in range(2)]
                        for i in range(NSK):
                            sci = pa.tile([P, 2048], FP, tag="sc", name="sc")
                            for h in range(2):
                                r0 = h * 64
                                for n in range(2):
                                    nc.tensor.matmul(
                                        sci[:, h * 1024 + n * 512:
                                            h * 1024 + (n + 1) * 512],
                                        kT[r0:r0 + 64, i * P:(i + 1) * P],
                                        qT[r0:r0 + 64,
                                           o + n * 512:o + (n + 1) * 512],
                                        start=True, stop=True)
                            e = epool.tile([P, 2048], FP, tag="e", name="e")
                            nc.scalar.activation(e[:], sci[:], Exp, scale=SCALE)
                            d0 = i * P - o
                            if 0 <= d0 < 1024:
                                kp = kpo.tile([P, P], FP, tag="kp", name="kp")
                                nc.vector.tensor_scalar_max(kp[:], tri_sb[:],
                                                            pad_sb[:, i:i + 1])
                                for h in range(2):
                                    hb = h * 1024
                                    nc.vector.tensor_mul(
                                        e[:, hb + d0:hb + d0 + P],
                                        e[:, hb + d0:hb + d0 + P], kp[:])
                            bnd = min(max(d0, 0), 1024)
                            for h in range(2):
                                hb = h * 1024
                                vsl = slice(i * 130 + g * 65,
                                            i * 130 + g * 65 + 65)
                                for w in range(2):
                                    c0, c1 = w * 512, (w + 1) * 512
                                    if bnd <= c0:
                                        segs = [(c0, c1, vv)]
                                    elif bnd >= c1:
                                        segs = [(c0, c1, vp)]
                                    else:
                                        segs = [(c0, bnd, vp), (bnd, c1, vv)]
                                    for (a, bcol, vt) in segs:
                                        nc.tensor.matmul(
                                            av[h][:, a:bcol], vt[:, vsl],
                                            e[:, hb + a:hb + bcol],
                                            start=(i == 0), stop=(i == NSK - 1),
                                            skip_group_check=True)
                        # normalize + stage for AllGather (lane-aligned)
                        nrm = npo.tile([65, 2048], FP, tag="nrm", name="nrm")
                        rc = npo.tile([1, 2048], FP, tag="rc", name="rc")
                        bcs = npo.tile([64, 2048], FP, tag="bcs", name="bcs")
                        st = stp.tile([64, 2048], FP, tag="st", name="st")
                        for h in range(2):
                            nc.vector.tensor_copy(
                                nrm[64:65, h * 1024:(h + 1) * 1024],
                                av[h][64:65, :])
                        nc.sync.dma_start(rc[0:1, :], nrm[64:65, :])
                        nc.vector.reciprocal(rc[0:1, :], rc[0:1, :])
                        bc = pa.tile([64, 2048], FP, tag="sc", name="bc")
                        for h in range(2):
                            for n in range(2):
                                cc = h * 1024 + n * 512
                                nc.tensor.matmul(bc[:, cc:cc + 512],
                                                 ones_sb[:, 0:64],
                                                 rc[:, cc:cc + 512],
                                                 start=True, stop=True)
                            nc.vector.tensor_copy(
                                bcs[:, h * 1024:(h + 1) * 1024],
                                bc[:, h * 1024:(h + 1) * 1024])
                            nc.vector.tensor_mul(
                                st[:, h * 1024:(h + 1) * 1024],
                                av[h][0:64, :],
                                bcs[:, h * 1024:(h + 1) * 1024])
                            nc.sync.dma_start(
                                ag_in[g * P + h * 64:g * P + (h + 1) * 64,
                                      o:o + 1024],
                                st[:, h * 1024:(h + 1) * 1024])

                # ---------- phase 3: AllGather + output projection ----------
                nc.gpsimd.collective_compute(
                    "AllGather", mybir.AluOpType.bypass,
                    replica_groups=[[0, 1, 2, 3], [4, 5, 6, 7]],
                    ins=[ag_in.opt()], outs=[ag_out.opt()])
                agsb = pp.tile([P, NK * S], FP, tag="xT")  # reuse xT slot
                for kk in range(NK):
                    nc.sync.dma_start(agsb[:, kk * S:(kk + 1) * S],
                                      ag_out[kk * P:(kk + 1) * P, :])
                with tc.tile_pool(name="out_pool", bufs=3) as op:
                    for m in range(S // P):
                        po = pa.tile([P, NSLICE], FP, tag=f"av{m % 2}",
                                     name="po")
                        nc.tensor.matmul(po[:], ones_sb[:, 0:P], bo_sb[:],
                                         start=True, stop=False)
                        for kk in range(NK):
                            nc.tensor.matmul(
                                po[:],
                                agsb[:, kk * S + m * P:kk * S + (m + 1) * P],
                                wo_sb[:, kk * NSLICE:(kk + 1) * NSLICE],
                                start=False, stop=(kk == NK - 1))
                        osb = op.tile([P, NSLICE], FP, tag="osb", name="osb")
                        nc.vector.tensor_copy(osb[:], po[:])
                        nc.sync.dma_start(outp[m * P:(m + 1) * P, :], osb[:])

    nc.compile()
    return nc


def _prep_core_inputs(x, Wq, bq, Wk, bk, Wv, bv, Wo, bo, freqs_cos, freqs_sin,
                      attention_mask, b, j):
    """Host-side slicing/permutation for core (b, j). Pure layout prep."""
    f32 = np.float32
    xT = np.ascontiguousarray(np.asarray(x[b], f32).T)  # [D, S]

    def head_cols(W, h, dh):
        cols = W[:, h * dh:(h + 1) * dh]
        return np.concatenate([cols[:, 0::2], cols[:, 1::2]], axis=1)

    def head_vec(v, h, dh):
        seg = v[h * dh:(h + 1) * dh]
        return np.concatenate([seg[0::2], seg[1::2]])

    qh = [4 * j + t for t in range(4)]
    kvh = [2 * j, 2 * j + 1]
    wq_p = np.concatenate([head_cols(np.asarray(Wq, f32), h, DK) for h in qh],
                          axis=1)
    wk_p = np.concatenate([head_cols(np.asarray(Wk, f32), h, DK) for h in kvh],
                          axis=1)
    Wv_ = np.asarray(Wv, f32)
    zero = np.zeros((D, 1), f32)
    wv_p = np.concatenate([Wv_[:, kvh[0] * DV:(kvh[0] + 1) * DV], zero,
                           Wv_[:, kvh[1] * DV:(kvh[1] + 1) * DV], zero], axis=1)
    bv_ = np.asarray(bv, f32)
    ev = np.concatenate([bv_[kvh[0] * DV:(kvh[0] + 1) * DV], [1.0],
                         bv_[kvh[1] * DV:(kvh[1] + 1) * DV], [1.0]]
                        ).astype(f32).reshape(1, 130)
    bq2 = np.concatenate([head_vec(np.asarray(bq, f32), h, DK)
                          for h in qh]).reshape(1, 2 * P)
    bk1 = np.concatenate([head_vec(np.asarray(bk, f32), h, DK)
                          for h in kvh]).reshape(1, P)
    wo_p = np.ascontiguousarray(
        np.asarray(Wo, f32)[:, j * NSLICE:(j + 1) * NSLICE])
    bo_n = np.asarray(bo, f32)[j * NSLICE:(j + 1) * NSLICE].reshape(1, NSLICE)
    cosA = np.tile(np.asarray(freqs_cos, f32).T, (4, 1))  # [128, S]
    sinA = np.tile(np.asarray(freqs_sin, f32).T, (4, 1))
    pad = np.asarray(attention_mask[b]).astype(f32)  # [S], 1 = real token
    tri = (np.arange(P)[None, :] >= np.arange(P)[:, None]).astype(f32)
    pad2 = np.ascontiguousarray(pad.reshape(16, P).T)  # pad2[r,i]=pad[i*128+r]
    ones = np.ones((1, S), f32)
    return {
        "xT": xT, "wq": wq_p, "wk": wk_p, "wv": wv_p, "ev": ev,
        "wo": wo_p, "bo": bo_n, "cosA": cosA, "sinA": sinA, "tri": tri,
        "pad2": pad2, "ones": ones, "bq2": bq2, "bk1": bk1,
    }


_NC_CACHE = {}


def _get_nc():
    if "nc" not in _NC_CACHE:
        _NC_CACHE["nc"] = build_nc()
    return _NC_CACHE["nc"]


def kernel(x, Wq, bq, Wk, bk, Wv, bv, Wo, bo, freqs_cos, freqs_sin,
           attention_mask):
    nc = _get_nc()
    in_maps = []
    for c in range(N_CORES):
        b, j = c // 4, c % 4
        in_maps.append(_prep_core_inputs(x, Wq, bq, Wk, bk, Wv, bv, Wo, bo,
                                         freqs_cos, freqs_sin, attention_mask,
                                         b, j))
    res = run_bass_kernel_spmd(nc, in_maps, core_ids=list(range(N_CORES)))
    out = np.empty((B, S, D), np.float32)
    for c in range(N_CORES):
        b, j = c // 4, c % 4
        out[b, :, j * NSLICE:(j + 1) * NSLICE] = res.results[c]["outp"]
    return out


if __name__ == "__main__":
    rng = np.random.default_rng(0)
    ins = {
        "x": rng.standard_normal((B, S, D), dtype=np.float32),
        "Wq": rng.standard_normal((D, H_Q * DK), dtype=np.float32) * 0.02,
        "bq": np.zeros(H_Q * DK, np.float32),
        "Wk": rng.standard_normal((D, H_KV * DK), dtype=np.float32) * 0.02,
        "bk": np.zeros(H_KV * DK, np.float32),
        "Wv": rng.standard_normal((D, H_KV * DV), dtype=np.float32) * 0.02,
        "bv": np.zeros(H_KV * DV, np.float32),
        "Wo": rng.standard_normal((H_Q * DV, D), dtype=np.float32) * 0.02,
        "bo": np.zeros(D, np.float32),
        "freqs_cos": rng.standard_normal((S, DK // 2), dtype=np.float32),
        "freqs_sin": rng.standard_normal((S, DK // 2), dtype=np.float32),
        "attention_mask": rng.random((B, S)) < 0.9,
    }
    out = kernel(**ins)
    print("ran, out shape", out.shape, "finite:", np.isfinite(out).all())
